# revision 14
# baseline (speedup 1.0000x reference)
"""ECC (edge-conditioned convolution) GNN message passing on 8 NeuronCores.

Strategy
--------
Edges are sorted by destination node (host side) and split into 8
contiguous, segment-aligned shards -- one per core.  The fnet's first two
layers (h1, h2) are evaluated on the host with BLAS; each core receives
h2 directly and runs, per 512-edge tile:

  PE    : 8 theta matmuls (4 PSUM pairs [128,2,512] f32) = W3T_b.T @ h2;
          fold passes reduce (o,i)-partitions to msg[o,e]:
            pair 0      -> 2 bf16 selector matmuls
            pairs 1,2,3 -> 1 fp8 DoubleRow matmul each (2 blocks/pass)
  ACT   : PSUM->SBUF evacuation of theta pairs 0,2 (f32->bf16) and of
          msg (f32->f32, feeds the Pool scan)
  DVE   : pair 0 bf16 prod mul (2x mode); pairs 1,3 muls straight from
          PSUM with fp8 output (1x)
  Pool  : pair 2 bf16->fp8 prod mul (SBUF only -- no PSUM port); masked
          prefix scan (tensor_tensor_scan) producing running segment sums

The scan output [32, E_c] goes back to HBM; the host reads each segment's
last column (positions are known statically from the sort), divides by
degree and applies the final relu.  Because shards are segment aligned no
cross-core reduction is needed.
"""

import math
import sys

import numpy as np

for _p in ("/opt/trn_rl_repo", "/root/.axon_site/_ro/trn_rl_repo"):
    if _p not in sys.path:
        sys.path.insert(0, _p)

import ml_dtypes

import concourse.bass as bass
import concourse.mybir as mybir
import concourse.tile as tile
from concourse import bacc
from concourse.bass_utils import run_bass_kernel_spmd

N_NODES = 25000
N_EDGES = 250000
F_IN = 32
F_OUT = 32
EDGE_DIM = 6
H1, H2 = 64, 128
N_CORES = 8
E_TILE = 512

# which theta pairs take which evac/mul path (see module docstring)
# act_* = ACT evacuates PSUM->bf16 first; dve_* = DVE muls straight from
# PSUM; *_bf16 folds via 2 bf16 selector matmuls, *_fp8 via 1 DoubleRow.
PAIR_PATH = ("act_dve_bf16", "act_dve_bf16", "act_dve_bf16", "dve_bf16")
POOL_SCAN = False

BF16 = ml_dtypes.bfloat16
F8E4 = ml_dtypes.float8_e4m3

_program_cache: dict = {}


def _build_program(
    e_c: int, bench_repeat: int | None = None, has_b3: bool = True
) -> "bass.Bass":
    f32 = mybir.dt.float32
    bf16 = mybir.dt.bfloat16
    f8e4 = mybir.dt.float8e4
    n_tiles = e_c // E_TILE

    n_bf = sum(p.endswith("bf16") for p in PAIR_PATH)

    nc = bacc.Bacc(None, target_bir_lowering=False)

    h2_d = nc.declare_dram_parameter("h2T", [H2, e_c], bf16, isOutput=False)
    xs_d = nc.declare_dram_parameter("xsrep", [128, e_c], bf16, isOutput=False)
    mk_d = nc.declare_dram_parameter("mask", [F_OUT, e_c], bf16, isOutput=False)
    w3_d = nc.declare_dram_parameter("w3T", [H2, F_OUT * F_IN], bf16, isOutput=False)
    sel_d = nc.declare_dram_parameter(
        "sel", [128, max(2 * n_bf, 1) * F_OUT], bf16, isOutput=False
    )
    sp8_d = nc.declare_dram_parameter(
        "selp8", [128, max(2 * (4 - n_bf), 1) * F_OUT], f8e4, isOutput=False
    )
    b3_d = nc.declare_dram_parameter("b3m", [F_IN, F_OUT], bf16, isOutput=False)
    out_d = nc.declare_dram_parameter("scan_out", [F_OUT, e_c], f32, isOutput=True)

    copy = mybir.ActivationFunctionType.Copy

    with tile.TileContext(nc) as tc:
        with (
            tc.tile_pool(name="const", bufs=1) as const,
            tc.tile_pool(name="io", bufs=4) as io,
            tc.tile_pool(name="mid", bufs=4) as mid,
            tc.tile_pool(name="scanb", bufs=4) as scanb,
            tc.tile_pool(name="psTH", bufs=3, space="PSUM") as psTH,
            tc.tile_pool(name="psMSG", bufs=2, space="PSUM") as psMSG,
        ):
            s_w3 = const.tile([H2, F_OUT * F_IN], bf16)
            nc.sync.dma_start(out=s_w3, in_=w3_d[:])
            s_sel = const.tile([128, max(2 * n_bf, 1) * F_OUT], bf16)
            nc.sync.dma_start(out=s_sel, in_=sel_d[:])
            s_sp8 = const.tile([128, max(2 * (4 - n_bf), 1) * F_OUT], f8e4)
            nc.sync.dma_start(out=s_sp8, in_=sp8_d[:])
            s_b3 = const.tile([F_IN, F_OUT], bf16)
            nc.sync.dma_start(out=s_b3, in_=b3_d[:])

            import contextlib

            loop_cm = (
                tc.For_i(
                    0,
                    bench_repeat,
                    1,
                    hint_engines=(
                        mybir.EngineType.PE,
                        mybir.EngineType.Activation,
                        mybir.EngineType.DVE,
                        mybir.EngineType.SP,
                        mybir.EngineType.Pool,
                    ),
                )
                if bench_repeat is not None
                else contextlib.nullcontext()
            )
            with loop_cm:
                prev_scan = None
                for t in range(n_tiles):
                    lo = t * E_TILE
                    hi = lo + E_TILE

                    h2_t = io.tile([H2, E_TILE], bf16, tag="h2")
                    nc.sync.dma_start(out=h2_t, in_=h2_d[:, lo:hi])
                    xs_t = io.tile([128, E_TILE], bf16, tag="xs")
                    nc.sync.dma_start(out=xs_t, in_=xs_d[:, lo:hi])
                    mk_t = io.tile([F_OUT, E_TILE], bf16, tag="mk")
                    nc.sync.dma_start(out=mk_t, in_=mk_d[:, lo:hi])

                    msgp = psMSG.tile([F_OUT, E_TILE], f32, tag="msg")
                    if has_b3:
                        nc.tensor.matmul(
                            msgp, s_b3, xs_t[0:F_IN, :], start=True, stop=False
                        )
                    # xs broadcast over the theta-pair dim (stride-0 middle)
                    xs2 = bass.AP(
                        tensor=xs_t.tensor,
                        offset=xs_t.offset,
                        ap=[list(xs_t.ap[0]), [0, 2], list(xs_t.ap[1])],
                    )
                    i_bf = 0
                    i_p8 = 0
                    first_fold = not has_b3
                    for p in range(4):
                        path = PAIR_PATH[p]
                        thp2 = psTH.tile([128, 2, E_TILE], f32, tag="th")
                        for h in range(2):
                            b = 2 * p + h
                            nc.tensor.matmul(
                                thp2[:, h, :],
                                s_w3[:, b * 128 : (b + 1) * 128],
                                h2_t,
                                start=True,
                                stop=True,
                            )
                        fold_bf16 = path.endswith("bf16")
                        # source for the per-edge xs multiply
                        if path.startswith("act"):
                            ths2 = mid.tile([128, 2, E_TILE], bf16, tag="ths")
                            nc.scalar.activation(ths2, thp2, copy)
                            mul_in = ths2
                        else:
                            mul_in = thp2
                        mul_eng = nc.gpsimd if "pool" in path else nc.vector
                        if fold_bf16:
                            prod2 = mid.tile([128, 2, E_TILE], bf16, tag="prod")
                            mul_eng.tensor_mul(prod2, mul_in, xs2)
                            for h in range(2):
                                nc.tensor.matmul(
                                    msgp,
                                    s_sel[
                                        :,
                                        (2 * i_bf + h) * F_OUT : (2 * i_bf + h + 1)
                                        * F_OUT,
                                    ],
                                    prod2[:, h, :],
                                    start=(first_fold and h == 0),
                                    stop=(p == 3 and h == 1),
                                )
                            first_fold = False
                            i_bf += 1
                        else:
                            prod8 = mid.tile([128, 2, E_TILE], f8e4, tag="p8")
                            mul_eng.tensor_mul(prod8, mul_in, xs2)
                            base = s_sp8[:, i_p8 * 2 * F_OUT : (i_p8 + 1) * 2 * F_OUT]
                            sp8 = bass.AP(
                                tensor=base.tensor,
                                offset=base.offset,
                                ap=[list(base.ap[0]), [F_OUT, 2], [1, F_OUT]],
                            )
                            nc.tensor.matmul(
                                msgp,
                                sp8,
                                prod8,
                                start=first_fold,
                                stop=(p == 3),
                                perf_mode=mybir.MatmulPerfMode.DoubleRow,
                            )
                            first_fold = False
                            i_p8 += 1

                    sc = scanb.tile([F_OUT, E_TILE], f32, tag="scan")
                    initial = (
                        0.0 if prev_scan is None else prev_scan[:, E_TILE - 1 : E_TILE]
                    )
                    if POOL_SCAN:
                        msgs = mid.tile([F_OUT, E_TILE], f32, tag="msgs")
                        nc.scalar.activation(msgs, msgp, copy)
                        nc.gpsimd.tensor_tensor_scan(
                            sc,
                            mk_t,
                            msgs,
                            initial=initial,
                            op0=mybir.AluOpType.mult,
                            op1=mybir.AluOpType.add,
                        )
                    else:
                        nc.vector.tensor_tensor_scan(
                            sc,
                            mk_t,
                            msgp,
                            initial=initial,
                            op0=mybir.AluOpType.mult,
                            op1=mybir.AluOpType.add,
                        )
                    prev_scan = sc
                    nc.sync.dma_start(out=out_d[:, lo:hi], in_=sc)

    nc.finalize()
    return nc


def _prepare(x, edge_attr, W1, b1, W2, b2, W3, b3, edge_src, edge_dst):
    x = np.asarray(x, dtype=np.float32)
    edge_attr = np.asarray(edge_attr, dtype=np.float32)
    W1 = np.asarray(W1, dtype=np.float32)
    b1 = np.asarray(b1, dtype=np.float32)
    W2 = np.asarray(W2, dtype=np.float32)
    b2 = np.asarray(b2, dtype=np.float32)
    W3 = np.asarray(W3, dtype=np.float32)
    b3 = np.asarray(b3, dtype=np.float32)
    edge_src = np.asarray(edge_src).astype(np.int64)
    edge_dst = np.asarray(edge_dst).astype(np.int64)

    n_nodes = x.shape[0]
    n_edges = edge_dst.shape[0]

    # ---- host preprocessing: sort by destination, shard on segment bounds
    order = np.argsort(edge_dst, kind="stable")
    dst_s = edge_dst[order]
    src_s = edge_src[order]
    ea_s = edge_attr[order]

    cuts = [0]
    for c in range(1, N_CORES):
        t = c * n_edges // N_CORES
        while t < n_edges and dst_s[t] == dst_s[t - 1]:
            t += 1
        cuts.append(min(t, n_edges))
    cuts.append(n_edges)
    counts = [cuts[i + 1] - cuts[i] for i in range(N_CORES)]
    e_c = max(E_TILE, int(math.ceil(max(counts) / E_TILE)) * E_TILE)

    deg = np.bincount(edge_dst, minlength=n_nodes).astype(np.float32)
    inv_deg = 1.0 / np.maximum(deg, 1.0)

    # ---- fnet first two layers on host (BLAS)
    h1 = np.maximum(ea_s @ W1.T + b1, 0.0)
    h2 = np.maximum(h1 @ W2.T + b2, 0.0)  # [E, 128] f32

    # ---- shared weight payloads
    w3T = np.ascontiguousarray(W3.T).astype(BF16)                  # [128, 1024]
    b3m = np.ascontiguousarray(b3.reshape(F_OUT, F_IN).T).astype(BF16)
    sel = np.zeros((128, 8 * F_OUT), dtype=np.float32)
    rows = np.arange(128)
    for b in range(8):
        sel[rows, b * F_OUT + (4 * b + rows // 32)] = 1.0
    bf_blocks = [2 * p + h for p in range(4) if PAIR_PATH[p].endswith("bf16")
                 for h in range(2)]
    p8_blocks = [2 * p + h for p in range(4) if not PAIR_PATH[p].endswith("bf16")
                 for h in range(2)]
    sel_bf = (
        np.concatenate(
            [sel[:, b * F_OUT : (b + 1) * F_OUT] for b in bf_blocks], axis=1
        ).astype(BF16)
        if bf_blocks
        else np.zeros((128, F_OUT), dtype=BF16)
    )
    sel_p8 = (
        np.concatenate(
            [sel[:, b * F_OUT : (b + 1) * F_OUT] for b in p8_blocks], axis=1
        ).astype(F8E4)
        if p8_blocks
        else np.zeros((128, F_OUT), dtype=F8E4)
    )

    in_maps = []
    core_meta = []
    for c in range(N_CORES):
        lo, hi = cuts[c], cuts[c + 1]
        cnt = hi - lo
        dst_c = dst_s[lo:hi]
        xs_c = x[src_s[lo:hi]]                                     # [cnt, 32]

        h2_pad = np.zeros((e_c, H2), dtype=np.float32)
        h2_pad[:cnt] = h2[lo:hi]
        xs_pad = np.zeros((e_c, F_IN), dtype=np.float32)
        xs_pad[:cnt] = xs_c
        keep = np.zeros(e_c, dtype=np.float32)
        if cnt > 1:
            keep[1:cnt] = (dst_c[1:] == dst_c[:-1]).astype(np.float32)

        h2T = np.ascontiguousarray(h2_pad.T).astype(BF16)          # [128, e_c]
        xsT = np.ascontiguousarray(xs_pad.T)                       # [32, e_c]
        xsrep = np.tile(xsT, (4, 1)).astype(BF16)                  # [128, e_c]
        mask = np.broadcast_to(keep, (F_OUT, e_c)).astype(BF16)

        # last index of each segment in this shard
        if cnt > 0:
            is_end = np.empty(cnt, dtype=bool)
            is_end[-1] = True
            is_end[:-1] = dst_c[1:] != dst_c[:-1]
            ends = np.flatnonzero(is_end)
            nodes = dst_c[ends]
        else:
            ends = np.zeros(0, dtype=np.int64)
            nodes = np.zeros(0, dtype=np.int64)
        core_meta.append((ends, nodes))

        in_maps.append(
            {
                "h2T": h2T,
                "xsrep": xsrep,
                "mask": np.ascontiguousarray(mask),
                "w3T": w3T,
                "sel": sel_bf,
                "selp8": sel_p8,
                "b3m": b3m,
            }
        )

    has_b3 = bool(np.any(b3))
    return {
        "in_maps": in_maps,
        "core_meta": core_meta,
        "inv_deg": inv_deg,
        "e_c": e_c,
        "has_b3": has_b3,
        "n_nodes": n_nodes,
    }


def _postprocess(res, meta):
    out = np.zeros((meta["n_nodes"], F_OUT), dtype=np.float32)
    inv_deg = meta["inv_deg"]
    for c in range(N_CORES):
        scan = np.asarray(res.results[c]["scan_out"], dtype=np.float32)
        ends, nodes = meta["core_meta"][c]
        if len(nodes):
            out[nodes] = scan[:, ends].T * inv_deg[nodes, None]
    np.maximum(out, 0.0, out=out)
    return out


def kernel(x, edge_attr, W1, b1, W2, b2, W3, b3, edge_src, edge_dst):
    meta = _prepare(x, edge_attr, W1, b1, W2, b2, W3, b3, edge_src, edge_dst)
    key = (meta["e_c"], meta["has_b3"])
    if key not in _program_cache:
        _program_cache[key] = _build_program(meta["e_c"], has_b3=meta["has_b3"])
    nc = _program_cache[key]

    res = run_bass_kernel_spmd(nc, meta["in_maps"], list(range(N_CORES)))
    return _postprocess(res, meta)
